# revision 52
# baseline (speedup 1.0000x reference)
"""Bass/Tile TRN2 kernel for nn_MultiHeadAttention_9277129359942.

B=2, T=S=2048, D=1024, H=16 heads, head_dim=64, fp32 I/O.

Sharding (8 cores): data-parallel over batch (2) x tensor-parallel over
head groups (4 heads / core, 256 out dims).  Each core computes the
attention for its 4 heads and a partial output projection; the host sums
the 4 partials per batch (row-parallel Wo) and adds bo once.

Device-side layout (transpose-free):
  - activations arrive feature-major, pre-cast:  x^T [D, T] bf16
  - weights arrive as W^T slices in bf16: wq/wk/wv [1024, 256], wo [256, 1024]
  - q,k produced transposed ([256, 2048], head dim on partitions); v in
    natural [S, 256] layout with a ones column per head (v_aug) so the
    attention's second matmul also produces the softmax denominator.
  - scores computed transposed (s on partitions, t free); softmax skips
    max-subtraction (scores ~ N(0,1), exp cannot overflow fp32/bf16).

Schedule: input DMAs are three whole-tensor transfers on the sync HWDGE
ring (xv, xk, xq) with weights on the scalar ring; projections run in
the shadow of the loads (v, k-m0, q-m0, k-m1, q-m1, double-buffered
4-bank PSUM); the attention loop then owns all 8 PSUM banks (2x sc
double-buffered + 2x ctx accumulators) and is Scalar-engine-bound on
exp.  A fraction of the exp tiles can be offloaded to the DVE using a
bf16 Schraudolph bit-trick (EXP_APPROX_FRAC).  Softmax normalization
uses reciprocal_approx_fast and a log2 DMA broadcast chain.  The output
projection runs at the tail with PSUM eviction alternating DVE/Scalar,
bf16 output, bo added on host.
"""

import os
import sys

import numpy as np

for _p in ("/opt/trn_rl_repo",):
    if os.path.isdir(_p) and _p not in sys.path:
        sys.path.append(_p)

import ml_dtypes

import concourse.bass as bass
import concourse.mybir as mybir
import concourse.tile as tile
from concourse import bacc
from concourse.bass_utils import run_bass_kernel_spmd

F32 = mybir.dt.float32
BF16 = mybir.dt.bfloat16
I16 = mybir.dt.int16
AF = mybir.ActivationFunctionType
ALU = mybir.AluOpType
BF16_NP = ml_dtypes.bfloat16

D = 1024          # model dim
T = 2048          # query length
S = 2048          # key length
P = 128           # partitions
KT = D // P       # 8 contraction tiles
TT = T // P       # 16 row tiles
ST = S // P       # 16 key tiles
HL = 4            # local heads per core
HD = 64           # head dim
OUTL = HL * HD    # 256 local out dims
VW = HD + 1       # v_aug width per head: [64 v | 1 ones].  A single
                  # ones column keeps the ctx matmul at M=65 -- wider
                  # denominator replicas burn enough PE power to trip
                  # the chip-level throttle (observed: M=96/128 runs
                  # intermittently downclock PE to 1.2GHz / ACT to 1GHz)
N_CORES = 8

# Schraudolph bf16 exp on DVE for a fraction of the (s, i) tiles:
#   bits(exp(x*0.125)) ~= round(x * 0.125*128/ln2 + (16256 - C))
EXP_A = 0.125 * 128.0 / float(np.log(2.0))
EXP_C = 5.0
ESPL = 640       # exact-exp columns per tile; 1024-ESPL go to the DVE
# which (s, i) tiles go to the DVE (per block); tuned for error budget
APPROX_SET = None  # set below
DEBUG_EX = False


def make_approx_set(frac_num, frac_den):
    sel = set()
    for s in range(ST):
        for i in range(2):
            if ((s * 2 + i) * frac_num) % frac_den < frac_num:
                sel.add((s, i))
    return frozenset(sel)


APPROX_SET = frozenset()


def build_program(approx_set=None):
    if approx_set is None:
        approx_set = APPROX_SET
    nc = bacc.Bacc(
        "TRN2", target_bir_lowering=False, debug=False, enable_asserts=True,
        num_devices=N_CORES,
    )

    xq_d = nc.dram_tensor("xq", [D, T], BF16, kind="ExternalInput")
    xk_d = [nc.dram_tensor(f"xk{h}", [D // 2, S], BF16,
                           kind="ExternalInput") for h in range(2)]
    xv_d = [nc.dram_tensor(f"xv{h}", [D // 2, S], BF16,
                           kind="ExternalInput") for h in range(2)]
    wq_d = nc.dram_tensor("wq", [D, OUTL], BF16, kind="ExternalInput")
    wk_d = nc.dram_tensor("wk", [D, OUTL], BF16, kind="ExternalInput")
    wv_d = nc.dram_tensor("wv", [D, OUTL], BF16, kind="ExternalInput")
    wo_d = nc.dram_tensor("wo", [OUTL, D], BF16, kind="ExternalInput")
    # biases packed host-side into final SBUF layout (single DMA with
    # contiguous per-partition runs; per-element descriptors are ~10us)
    # cols 0:2 = bq[m], 2:4 = bk[m], 4:516 = bv broadcast, twice
    bias_d = nc.dram_tensor("bias_pack", [P, 4 + 2 * OUTL], F32,
                            kind="ExternalInput")
    out_d = nc.dram_tensor("out", [T, D], BF16, kind="ExternalOutput")
    wsink_d = nc.dram_tensor("warm_sink", [1, 8], F32, kind="ExternalOutput")
    dbg_d = None
    if DEBUG_EX:
        dbg_d = nc.dram_tensor("dbg_ex", [P, 1024], BF16, kind="ExternalOutput")

    with tile.TileContext(nc) as tc:
        _build(nc, tc, xq_d, xk_d, xv_d, wq_d, wk_d, wv_d, wo_d,
               bias_d, out_d, wsink_d, approx_set, dbg_d)
    nc.compile()
    return nc


def _build(nc, tc, xq_d, xk_d, xv_d, wq_d, wk_d, wv_d, wo_d,
           bias_d, out_d, wsink_d, approx_set, dbg_d):
    from contextlib import ExitStack

    stack = ExitStack()
    with stack:
        consts = stack.enter_context(tc.tile_pool(name="consts", bufs=1))
        wpool = stack.enter_context(tc.tile_pool(name="wpool", bufs=1))
        acts = stack.enter_context(tc.tile_pool(name="acts", bufs=1))

        # ---- weights + biases on the scalar (ACT) HWDGE ring ----------
        wv_sb = wpool.tile([P, KT * OUTL], BF16, name="wv", tag="wv")
        wk_sb = wpool.tile([P, KT * OUTL], BF16, name="wk", tag="wk")
        wq_sb = wpool.tile([P, KT * OUTL], BF16, name="wq", tag="wq")
        wo_sb = wpool.tile([P, 2 * D], BF16, name="wo", tag="wo")
        bias_sb = consts.tile([P, 4 + 2 * OUTL], F32, name="bias",
                              tag="bias")

        def load_tiled(eng, sb, d_, inner):
            eng.dma_start(sb[:].rearrange("p (k o) -> p k o", o=inner),
                          d_.rearrange("(k p) o -> p k o", p=P))

        # x tensors split across both HWDGE rings: xv+xk on sync,
        # weights+biases then xq on scalar.
        xpool = stack.enter_context(tc.tile_pool(name="xpool", bufs=1))
        xv_sb = [xpool.tile([P, (KT // 2) * S], BF16, name=f"xv{h}",
                            tag=f"xv{h}") for h in range(2)]
        xk_sb = [xpool.tile([P, (KT // 2) * S], BF16, name=f"xk{h}",
                            tag=f"xk{h}") for h in range(2)]
        xq_sb = xpool.tile([P, KT * T], BF16, name="xq", tag="xq")

        def xcol(sb_pair, k, lo, hi):
            return sb_pair[k // 4][:, (k % 4) * S + lo:(k % 4) * S + hi]

        # xk first: the projection chain is k -> v -> q
        for h in range(2):
            load_tiled(nc.sync, xk_sb[h], xk_d[h], S)
        for h in range(2):
            load_tiled(nc.sync, xv_sb[h], xv_d[h], S)
        load_tiled(nc.scalar, wv_sb, wv_d, OUTL)
        nc.scalar.dma_start(bias_sb[:], bias_d[:, :])
        load_tiled(nc.scalar, wk_sb, wk_d, OUTL)
        load_tiled(nc.scalar, wq_sb, wq_d, OUTL)
        load_tiled(nc.scalar, wo_sb, wo_d, D)
        # xq last: its transfer would otherwise steal half the HBM
        # bandwidth from the xk/xv stream that gates the projections
        load_tiled(nc.scalar, xq_sb, xq_d, T)

        # persistent activations
        qT = [acts.tile([P, T], BF16, name=f"qT{m}", tag=f"qT{m}")
              for m in range(2)]
        kT = [acts.tile([P, S], BF16, name=f"kT{m}", tag=f"kT{m}")
              for m in range(2)]
        v_aug = acts.tile([P, ST * HL * VW], BF16, name="vaug", tag="vaug")
        ctxT = [[acts.tile([P, 1024], BF16, name=f"ctxT{p}{th}",
                           tag=f"ctxT{p}{th}") for th in range(2)]
                for p in range(2)]
        nc.vector.memset(v_aug[:], 1.0)  # ones columns survive the v writes

        # ---- warmup, then k -> v -> q projections ---------------------
        # k first (xk is loaded first), so the PE chain starts as soon
        # as the first tensor lands; q last (xq arrives on the scalar
        # ring behind the weights).
        with tc.tile_pool(name="wpsum", bufs=2, space="PSUM") as wpsum:
            # HAM warmup: dense matmul burst on (not-yet-written) kT/qT
            # tiles to un-throttle the PE clock; sunk to an output so it
            # is not dead-code-eliminated.
            warm_ps = None
            for grp in range(2):
                warm_ps = wpsum.tile([P, 512], F32, name="warm", tag="warm")
                for w in range(8):
                    nc.tensor.matmul(warm_ps[:], kT[0][0:HD, 0:P],
                                     qT[0][0:HD, 0:512],
                                     start=(w == 0), stop=(w == 7))
            wsnk = consts.tile([1, 8], F32, name="wsnk", tag="wsnk")
            nc.vector.tensor_copy(wsnk[:], warm_ps[0:1, 0:8])
            nc.gpsimd.dma_start(wsink_d[:, :], wsnk[:])

        def qk_phase(qkpsum, w_sb, x_sb, boff, o_sb, m):
            # single 1-bank psum buffer: every chunk WAR-waits on the
            # previous chunk's eviction, pacing the PE at ~65% duty -- a
            # sustained full-array matmul burst trips the chip power
            # governor, which then clamps the PE to half clock for
            # ~250us (costing far more than the pacing does).
            ps = qkpsum.tile([P, 512], F32, name=f"pqk{m}", tag="pqk")
            for c in range(4):
                cs = slice(0, 512)
                for k in range(KT):
                    if x_sb is xq_sb:
                        xap = x_sb[:, k * T + c * 512: k * T + (c + 1) * 512]
                    else:
                        xap = xcol(x_sb, k, c * 512, (c + 1) * 512)
                    nc.tensor.matmul(
                        ps[:, cs],
                        w_sb[:, k * OUTL + m * P: k * OUTL + (m + 1) * P],
                        xap,
                        start=(k == 0), stop=(k == KT - 1))
                nc.vector.tensor_scalar_add(
                    o_sb[m][:, c * 512:(c + 1) * 512], ps[:, cs],
                    bias_sb[:, boff + m: boff + m + 1])

        with tc.tile_pool(name="kpsum", bufs=1, space="PSUM") as kpsum:
            for m in range(2):
                qk_phase(kpsum, wk_sb, xk_sb, 2, kT, m)

        with tc.tile_pool(name="vpsum", bufs=2, space="PSUM") as vpsum:
            bv3 = bias_sb[:, 4:4 + OUTL].rearrange("p (h x) -> p h x", x=HD)
            for s in range(ST):
                ps = vpsum.tile([P, OUTL], F32, name="pv", tag="pv")
                for k in range(KT):
                    nc.tensor.matmul(
                        ps[:], xcol(xv_sb, k, s * P, (s + 1) * P),
                        wv_sb[:, k * OUTL:(k + 1) * OUTL],
                        start=(k == 0), stop=(k == KT - 1))
                dst = v_aug[:, s * HL * VW:(s + 1) * HL * VW]
                dst = dst.rearrange("p (h x) -> p h x", x=VW)[:, :, 0:HD]
                nc.vector.tensor_tensor(
                    out=dst, in0=ps[:].rearrange("p (h x) -> p h x", x=HD),
                    in1=bv3, op=ALU.add)

        with tc.tile_pool(name="qpsum", bufs=1, space="PSUM") as qpsum:
            for m in range(2):
                qk_phase(qpsum, wq_sb, xq_sb, 0, qT, m)

        # ---- attention ------------------------------------------------
        with tc.tile_pool(name="spsum", bufs=1, space="PSUM") as spsum, \
             tc.tile_pool(name="cpsum", bufs=1, space="PSUM") as cpsum, \
             tc.tile_pool(name="epool", bufs=2) as epool, \
             tc.tile_pool(name="npool", bufs=1) as npool, \
             tc.tile_pool(name="opool", bufs=4) as opool:

            def emit_norm(p, th, ctx_ps, last=False):
                """ctx psum rows 0:64 hold the unnormalized ctx, row 64
                the softmax denominator.  Evict to SBUF (frees the psum
                banks for the next block), replicate the denominator row
                to 32 base-0 rows via a doubling DMA chain, take the
                fast reciprocal wide, fill rows 32:64, and multiply.
                Head 0 lands directly in ctxT rows 0:64; head 1 goes via
                a staging tile (DVE lanes are partition-locked).  The
                final block skips the staging copy to shorten the tail,
                except the denominator row (DMA cannot read PSUM)."""
                for i in range(2):
                    stg = npool.tile([P, 1024], F32, name=f"stg{i}",
                                     tag=f"stg{i}")
                    if last:
                        nc.vector.tensor_copy(stg[HD:VW, :],
                                              ctx_ps[i][HD:VW, :])
                        cin = ctx_ps[i]
                    else:
                        nc.vector.tensor_copy(stg[0:VW, :],
                                              ctx_ps[i][0:VW, :])
                        cin = stg
                    rb = npool.tile([P, 1024], F32, name=f"rb{i}",
                                    tag=f"rb{i}")
                    rd = npool.tile([P, 1024], F32, name=f"rd{i}",
                                    tag=f"rd{i}")
                    nc.sync.dma_start(rd[0:1, :], stg[HD:VW, :])
                    w = 1
                    while w < 32:
                        nc.sync.dma_start(rd[w:2 * w, :], rd[0:w, :])
                        w *= 2
                    nc.vector.reciprocal_approx_fast(rb[0:32, :],
                                                     rd[0:32, :])
                    nc.sync.dma_start(rb[32:HD, :], rb[0:32, :])
                    if i == 0:
                        nc.vector.tensor_tensor(
                            out=ctxT[p][th][0:HD, :], in0=cin[0:HD, :],
                            in1=rb[0:HD, :], op=ALU.mult)
                    else:
                        ostg = npool.tile([P, 1024], BF16, name="ostg",
                                          tag="ostg")
                        nc.vector.tensor_tensor(
                            out=ostg[0:HD, :], in0=cin[0:HD, :],
                            in1=rb[0:HD, :], op=ALU.mult)
                        nc.sync.dma_start(ctxT[p][th][HD:P, :],
                                          ostg[0:HD, :])

            def emit_scores(p, th, s, i):
                sc = spsum.tile([P, 1024], F32, name=f"sc{i}", tag=f"sc{i}")
                t0 = th * 1024
                ss = slice(s * P, (s + 1) * P)
                hp = slice(i * HD, (i + 1) * HD)
                for c in range(2):
                    nc.tensor.matmul(
                        sc[:, c * 512:(c + 1) * 512], kT[p][hp, ss],
                        qT[p][hp, t0 + c * 512:t0 + (c + 1) * 512],
                        start=True, stop=True)
                return sc

            def emit_ctx(ctx_ps, p, s, i, e):
                h = 2 * p + i
                vs = slice(s * HL * VW + h * VW, s * HL * VW + h * VW + VW)
                for c in range(2):
                    nc.tensor.matmul(
                        ctx_ps[i][:, c * 512:(c + 1) * 512], v_aug[:, vs],
                        e[:, c * 512:(c + 1) * 512],
                        start=(s == 0), stop=(s == ST - 1))

            # software-pipelined, head-serial: the PE stream per step is
            # [scores(s+1,0), ctx(s,0), scores(s+1,1), ctx(s,1)] so no
            # waiting instruction blocks ready work behind it, and the
            # exp stream stays back-to-back on the Scalar engine.
            blocks = ((0, 0), (1, 0), (0, 1), (1, 1))
            sc = [emit_scores(*blocks[0], 0, i) for i in range(2)]
            for bi, (p, th) in enumerate(blocks):
                ctx_ps = [cpsum.tile([VW, 1024], F32, name=f"ctx{i}",
                                     tag=f"ctx{i}") for i in range(2)]
                for s in range(ST):
                    ex = []
                    for i in range(2):
                        # exp split by columns: ScalarE does 0:ESPL
                        # exactly while the DVE computes the rest with
                        # the bf16 Schraudolph bit-trick.  The scores
                        # WAR releases ~0.8us earlier, shortening the
                        # chain exp -> scores(s+1) -> exp that bounds
                        # the loop.
                        e = epool.tile([P, 1024], BF16, name=f"ex{i}",
                                       tag=f"ex{i}")
                        nc.scalar.activation(e[:, 0:ESPL], sc[i][:, 0:ESPL],
                                             AF.Exp, scale=0.125)
                        nc.vector.tensor_scalar(
                            out=e[:, ESPL:1024].bitcast(I16),
                            in0=sc[i][:, ESPL:1024],
                            scalar1=EXP_A, scalar2=16256.0 - EXP_C,
                            op0=ALU.mult, op1=ALU.add)
                        if dbg_d is not None and (p, th, s, i) == (0, 0, 0, 0):
                            nc.sync.dma_start(dbg_d[:, :], e[:])
                        ex.append(e)
                    for i in range(2):
                        if s + 1 < ST:
                            sc[i] = emit_scores(p, th, s + 1, i)
                        elif bi + 1 < len(blocks):
                            sc[i] = emit_scores(*blocks[bi + 1], 0, i)
                        emit_ctx(ctx_ps, p, s, i, ex[i])
                emit_norm(p, th, ctx_ps, last=(bi == len(blocks) - 1))

            # ---- output projection (tail) -----------------------------
            # th=0 tiles evict via the (now idle) Scalar engine so the
            # DVE can finish the last block's normalize concurrently;
            # th=1 tiles alternate DVE/Scalar.  A keepalive matmul burst
            # into the first psum buffer keeps the PE busy through the
            # last normalize (a >3.4us PE idle would re-throttle the
            # clock to 1.2GHz for the whole projection); it is reset by
            # the projection's start=True.
            ka = spsum.tile([P, D], F32, name="ka", tag="sc0")
            for w in range(24):
                nc.tensor.matmul(ka[:, 0:512], ctxT[0][0][:, 0:P],
                                 wo_sb[:, 0:512],
                                 start=(w == 0), stop=(w == 23))
            for t in range(TT):
                th, tl = divmod(t, TT // 2)
                ts = slice(tl * P, (tl + 1) * P)
                if th == 0:
                    ps = spsum.tile([P, D], F32, name="po", tag=f"sc{t % 2}")
                else:
                    tags = ("sc0", "sc1", "ctx0", "ctx1")
                    pool = spsum if t % 4 < 2 else cpsum
                    ps = pool.tile([P, D], F32, name="po", tag=tags[t % 4])
                for p2 in range(2):
                    for n in range(2):
                        ns = slice(n * 512, (n + 1) * 512)
                        nc.tensor.matmul(
                            ps[:, ns], ctxT[p2][th][:, ts],
                            wo_sb[:, p2 * D + n * 512: p2 * D + (n + 1) * 512],
                            start=(p2 == 0), stop=(p2 == 1))
                ost = opool.tile([P, D], BF16, name="ost", tag="ost")
                if th == 0 or t % 2 == 1:
                    nc.scalar.activation(ost[:], ps[:], AF.Copy)
                else:
                    nc.vector.tensor_copy(ost[:], ps[:])
                rings = ((nc.sync, nc.gpsimd) if th == 0
                         else (nc.sync, nc.scalar, nc.gpsimd))
                rings[t % len(rings)].dma_start(
                    out_d[t * P:(t + 1) * P, :], ost[:])


def make_in_maps(query, key, value, Wq, bq, Wk, bk, Wv, bv, Wo, bo):
    """Shard the full inputs into the 8 per-core input dicts."""
    query, key, value, Wq, bq, Wk, bk, Wv, bv, Wo, bo = [
        np.asarray(a, dtype=np.float32)
        for a in (query, key, value, Wq, bq, Wk, bk, Wv, bv, Wo, bo)]

    def bf(a):
        return np.ascontiguousarray(a).astype(BF16_NP)

    in_maps = []
    for c in range(N_CORES):
        b, g = divmod(c, 4)
        sl = slice(g * OUTL, (g + 1) * OUTL)
        kT_ = key[b].T
        vT_ = value[b].T
        in_maps.append({
            "xq": bf(query[b].T),
            "xk0": bf(kT_[:D // 2]), "xk1": bf(kT_[D // 2:]),
            "xv0": bf(vT_[:D // 2]), "xv1": bf(vT_[D // 2:]),
            "wq": bf(Wq[sl, :].T),
            "wk": bf(Wk[sl, :].T),
            "wv": bf(Wv[sl, :].T),
            "wo": bf(Wo[:, sl].T),
            "bias_pack": np.concatenate([
                bq[sl].reshape(2, P).T, bk[sl].reshape(2, P).T,
                np.broadcast_to(bv[sl], (P, OUTL)),
                np.broadcast_to(bv[sl], (P, OUTL))], axis=1).copy(),
        })
    return in_maps


_NC_CACHE = None


def _get_nc():
    global _NC_CACHE
    if _NC_CACHE is None:
        _NC_CACHE = build_program()
    return _NC_CACHE


def gather_out(results, bo):
    out = np.empty((2, T, D), dtype=np.float32)
    bo = np.asarray(bo, dtype=np.float32)
    for b in range(2):
        acc = results[4 * b]["out"].astype(np.float32)
        for g in range(1, 4):
            acc = acc + results[4 * b + g]["out"].astype(np.float32)
        out[b] = acc + bo
    return out


def kernel(query, key, value, Wq, bq, Wk, bk, Wv, bv, Wo, bo):
    nc = _get_nc()
    in_maps = make_in_maps(query, key, value, Wq, bq, Wk, bk, Wv, bv, Wo, bo)
    res = run_bass_kernel_spmd(nc, in_maps, list(range(N_CORES)))
    return gather_out(res.results, bo)


# revision 53
# speedup vs baseline: 1.2483x; 1.2483x over previous
"""Bass/Tile TRN2 kernel for nn_MultiHeadAttention_9277129359942.

B=2, T=S=2048, D=1024, H=16 heads, head_dim=64, fp32 I/O.

Sharding (8 cores): data-parallel over batch (2) x tensor-parallel over
head groups (4 heads / core, 256 out dims).  Each core computes the
attention for its 4 heads and a partial output projection; the host sums
the 4 partials per batch (row-parallel Wo) and adds bo once.

Device-side layout (transpose-free):
  - activations arrive feature-major, pre-cast:  x^T [D, T] bf16
  - weights arrive as W^T slices in bf16: wq/wk/wv [1024, 256], wo [256, 1024]
  - q,k produced transposed ([256, 2048], head dim on partitions); v in
    natural [S, 256] layout with a ones column per head (v_aug) so the
    attention's second matmul also produces the softmax denominator.
  - scores computed transposed (s on partitions, t free); softmax skips
    max-subtraction (scores ~ N(0,1), exp cannot overflow fp32/bf16).

Schedule: input DMAs are three whole-tensor transfers on the sync HWDGE
ring (xv, xk, xq) with weights on the scalar ring; projections run in
the shadow of the loads (v, k-m0, q-m0, k-m1, q-m1, double-buffered
4-bank PSUM); the attention loop then owns all 8 PSUM banks (2x sc
double-buffered + 2x ctx accumulators) and is Scalar-engine-bound on
exp.  A fraction of the exp tiles can be offloaded to the DVE using a
bf16 Schraudolph bit-trick (EXP_APPROX_FRAC).  Softmax normalization
uses reciprocal_approx_fast and a log2 DMA broadcast chain.  The output
projection runs at the tail with PSUM eviction alternating DVE/Scalar,
bf16 output, bo added on host.
"""

import os
import sys

import numpy as np

for _p in ("/opt/trn_rl_repo",):
    if os.path.isdir(_p) and _p not in sys.path:
        sys.path.append(_p)

import ml_dtypes

import concourse.bass as bass
import concourse.mybir as mybir
import concourse.tile as tile
from concourse import bacc
from concourse.bass_utils import run_bass_kernel_spmd

F32 = mybir.dt.float32
BF16 = mybir.dt.bfloat16
I16 = mybir.dt.int16
AF = mybir.ActivationFunctionType
ALU = mybir.AluOpType
BF16_NP = ml_dtypes.bfloat16

D = 1024          # model dim
T = 2048          # query length
S = 2048          # key length
P = 128           # partitions
KT = D // P       # 8 contraction tiles
TT = T // P       # 16 row tiles
ST = S // P       # 16 key tiles
HL = 4            # local heads per core
HD = 64           # head dim
OUTL = HL * HD    # 256 local out dims
VW = HD + 1       # v_aug width per head: [64 v | 1 ones].  A single
                  # ones column keeps the ctx matmul at M=65 -- wider
                  # denominator replicas burn enough PE power to trip
                  # the chip-level throttle (observed: M=96/128 runs
                  # intermittently downclock PE to 1.2GHz / ACT to 1GHz)
N_CORES = 8

# Schraudolph bf16 exp on DVE for a fraction of the (s, i) tiles:
#   bits(exp(x*0.125)) ~= round(x * 0.125*128/ln2 + (16256 - C))
EXP_A = 0.125 * 128.0 / float(np.log(2.0))
EXP_C = 5.0
ESPL = 640       # exact-exp columns per tile; 1024-ESPL go to the DVE
# which (s, i) tiles go to the DVE (per block); tuned for error budget
APPROX_SET = None  # set below
DEBUG_EX = False


def make_approx_set(frac_num, frac_den):
    sel = set()
    for s in range(ST):
        for i in range(2):
            if ((s * 2 + i) * frac_num) % frac_den < frac_num:
                sel.add((s, i))
    return frozenset(sel)


APPROX_SET = frozenset()


def build_program(approx_set=None):
    if approx_set is None:
        approx_set = APPROX_SET
    nc = bacc.Bacc(
        "TRN2", target_bir_lowering=False, debug=False, enable_asserts=True,
        num_devices=N_CORES,
    )

    xq_d = nc.dram_tensor("xq", [D, T], BF16, kind="ExternalInput")
    xk_d = [nc.dram_tensor(f"xk{h}", [D // 2, S], BF16,
                           kind="ExternalInput") for h in range(2)]
    xv_d = [nc.dram_tensor(f"xv{h}", [D // 2, S], BF16,
                           kind="ExternalInput") for h in range(2)]
    wq_d = nc.dram_tensor("wq", [D, OUTL], BF16, kind="ExternalInput")
    wk_d = nc.dram_tensor("wk", [D, OUTL], BF16, kind="ExternalInput")
    wv_d = nc.dram_tensor("wv", [D, OUTL], BF16, kind="ExternalInput")
    wo_d = nc.dram_tensor("wo", [OUTL, D], BF16, kind="ExternalInput")
    # biases packed host-side into final SBUF layout (single DMA with
    # contiguous per-partition runs; per-element descriptors are ~10us)
    # cols 0:2 = bq[m], 2:4 = bk[m], 4:516 = bv broadcast, twice
    bias_d = nc.dram_tensor("bias_pack", [P, 4 + 2 * OUTL], F32,
                            kind="ExternalInput")
    out_d = nc.dram_tensor("out", [T, D], BF16, kind="ExternalOutput")
    wsink_d = nc.dram_tensor("warm_sink", [1, 8], F32, kind="ExternalOutput")
    dbg_d = None
    if DEBUG_EX:
        dbg_d = nc.dram_tensor("dbg_ex", [P, 1024], BF16, kind="ExternalOutput")

    with tile.TileContext(nc) as tc:
        _build(nc, tc, xq_d, xk_d, xv_d, wq_d, wk_d, wv_d, wo_d,
               bias_d, out_d, wsink_d, approx_set, dbg_d)
    nc.compile()
    return nc


def _build(nc, tc, xq_d, xk_d, xv_d, wq_d, wk_d, wv_d, wo_d,
           bias_d, out_d, wsink_d, approx_set, dbg_d):
    from contextlib import ExitStack

    stack = ExitStack()
    with stack:
        consts = stack.enter_context(tc.tile_pool(name="consts", bufs=1))
        wpool = stack.enter_context(tc.tile_pool(name="wpool", bufs=1))
        acts = stack.enter_context(tc.tile_pool(name="acts", bufs=1))

        # ---- weights + biases on the scalar (ACT) HWDGE ring ----------
        wv_sb = wpool.tile([P, KT * OUTL], BF16, name="wv", tag="wv")
        wk_sb = wpool.tile([P, KT * OUTL], BF16, name="wk", tag="wk")
        wq_sb = wpool.tile([P, KT * OUTL], BF16, name="wq", tag="wq")
        wo_sb = wpool.tile([P, 2 * D], BF16, name="wo", tag="wo")
        bias_sb = consts.tile([P, 4 + 2 * OUTL], F32, name="bias",
                              tag="bias")

        def load_tiled(eng, sb, d_, inner):
            eng.dma_start(sb[:].rearrange("p (k o) -> p k o", o=inner),
                          d_.rearrange("(k p) o -> p k o", p=P))

        # x tensors split across both HWDGE rings: xv+xk on sync,
        # weights+biases then xq on scalar.
        xpool = stack.enter_context(tc.tile_pool(name="xpool", bufs=1))
        xv_sb = [xpool.tile([P, (KT // 2) * S], BF16, name=f"xv{h}",
                            tag=f"xv{h}") for h in range(2)]
        xk_sb = [xpool.tile([P, (KT // 2) * S], BF16, name=f"xk{h}",
                            tag=f"xk{h}") for h in range(2)]
        xq_sb = xpool.tile([P, KT * T], BF16, name="xq", tag="xq")

        def xcol(sb_pair, k, lo, hi):
            return sb_pair[k // 4][:, (k % 4) * S + lo:(k % 4) * S + hi]

        # xk first: the projection chain is k -> v -> q
        for h in range(2):
            load_tiled(nc.sync, xk_sb[h], xk_d[h], S)
        for h in range(2):
            load_tiled(nc.sync, xv_sb[h], xv_d[h], S)
        load_tiled(nc.scalar, wv_sb, wv_d, OUTL)
        nc.scalar.dma_start(bias_sb[:], bias_d[:, :])
        load_tiled(nc.scalar, wk_sb, wk_d, OUTL)
        load_tiled(nc.scalar, wq_sb, wq_d, OUTL)
        load_tiled(nc.scalar, wo_sb, wo_d, D)
        # xq last: its transfer would otherwise steal half the HBM
        # bandwidth from the xk/xv stream that gates the projections
        load_tiled(nc.scalar, xq_sb, xq_d, T)

        # persistent activations
        qT = [acts.tile([P, T], BF16, name=f"qT{m}", tag=f"qT{m}")
              for m in range(2)]
        kT = [acts.tile([P, S], BF16, name=f"kT{m}", tag=f"kT{m}")
              for m in range(2)]
        v_aug = acts.tile([P, ST * HL * VW], BF16, name="vaug", tag="vaug")
        ctxT = [[acts.tile([P, 1024], BF16, name=f"ctxT{p}{th}",
                           tag=f"ctxT{p}{th}") for th in range(2)]
                for p in range(2)]
        nc.vector.memset(v_aug[:], 1.0)  # ones columns survive the v writes

        # ---- warmup, then k -> v -> q projections ---------------------
        # k first (xk is loaded first), so the PE chain starts as soon
        # as the first tensor lands; q last (xq arrives on the scalar
        # ring behind the weights).
        with tc.tile_pool(name="wpsum", bufs=2, space="PSUM") as wpsum:
            # HAM warmup: dense matmul burst on (not-yet-written) kT/qT
            # tiles to un-throttle the PE clock; sunk to an output so it
            # is not dead-code-eliminated.
            warm_ps = None
            for grp in range(2):
                warm_ps = wpsum.tile([P, 512], F32, name="warm", tag="warm")
                for w in range(8):
                    nc.tensor.matmul(warm_ps[:], kT[0][0:HD, 0:P],
                                     qT[0][0:HD, 0:512],
                                     start=(w == 0), stop=(w == 7))
            wsnk = consts.tile([1, 8], F32, name="wsnk", tag="wsnk")
            nc.vector.tensor_copy(wsnk[:], warm_ps[0:1, 0:8])
            nc.gpsimd.dma_start(wsink_d[:, :], wsnk[:])

        def qk_phase(qkpsum, w_sb, x_sb, boff, o_sb, m):
            # single 1-bank psum buffer: every chunk WAR-waits on the
            # previous chunk's eviction, pacing the PE at ~65% duty -- a
            # sustained full-array matmul burst trips the chip power
            # governor, which then clamps the PE to half clock for
            # ~250us (costing far more than the pacing does).
            ps = qkpsum.tile([P, 512], F32, name=f"pqk{m}", tag="pqk")
            for c in range(4):
                cs = slice(0, 512)
                for k in range(KT):
                    if x_sb is xq_sb:
                        xap = x_sb[:, k * T + c * 512: k * T + (c + 1) * 512]
                    else:
                        xap = xcol(x_sb, k, c * 512, (c + 1) * 512)
                    nc.tensor.matmul(
                        ps[:, cs],
                        w_sb[:, k * OUTL + m * P: k * OUTL + (m + 1) * P],
                        xap,
                        start=(k == 0), stop=(k == KT - 1))
                nc.vector.tensor_scalar_add(
                    o_sb[m][:, c * 512:(c + 1) * 512], ps[:, cs],
                    bias_sb[:, boff + m: boff + m + 1])

        with tc.tile_pool(name="kpsum", bufs=1, space="PSUM") as kpsum:
            for m in range(2):
                qk_phase(kpsum, wk_sb, xk_sb, 2, kT, m)

        with tc.tile_pool(name="vpsum", bufs=2, space="PSUM") as vpsum:
            bv3 = bias_sb[:, 4:4 + OUTL].rearrange("p (h x) -> p h x", x=HD)
            for s in range(ST):
                ps = vpsum.tile([P, OUTL], F32, name="pv", tag="pv")
                for k in range(KT):
                    nc.tensor.matmul(
                        ps[:], xcol(xv_sb, k, s * P, (s + 1) * P),
                        wv_sb[:, k * OUTL:(k + 1) * OUTL],
                        start=(k == 0), stop=(k == KT - 1))
                dst = v_aug[:, s * HL * VW:(s + 1) * HL * VW]
                dst = dst.rearrange("p (h x) -> p h x", x=VW)[:, :, 0:HD]
                nc.vector.tensor_tensor(
                    out=dst, in0=ps[:].rearrange("p (h x) -> p h x", x=HD),
                    in1=bv3, op=ALU.add)

        with tc.tile_pool(name="qpsum", bufs=1, space="PSUM") as qpsum:
            for m in range(2):
                qk_phase(qpsum, wq_sb, xq_sb, 0, qT, m)

        # ---- attention ------------------------------------------------
        with tc.tile_pool(name="spsum", bufs=1, space="PSUM") as spsum, \
             tc.tile_pool(name="cpsum", bufs=1, space="PSUM") as cpsum, \
             tc.tile_pool(name="epool", bufs=2) as epool, \
             tc.tile_pool(name="npool", bufs=1) as npool, \
             tc.tile_pool(name="opool", bufs=4) as opool:

            def emit_norm(p, th, ctx_ps, last=False):
                """ctx psum rows 0:64 hold the unnormalized ctx, row 64
                the softmax denominator.  Evict to SBUF (frees the psum
                banks for the next block), replicate the denominator row
                to 32 base-0 rows via a doubling DMA chain, take the
                fast reciprocal wide, fill rows 32:64, and multiply.
                Head 0 lands directly in ctxT rows 0:64; head 1 goes via
                a staging tile (DVE lanes are partition-locked).  The
                final block skips the staging copy to shorten the tail,
                except the denominator row (DMA cannot read PSUM)."""
                for i in range(2):
                    stg = npool.tile([P, 1024], F32, name=f"stg{i}",
                                     tag=f"stg{i}")
                    if last:
                        nc.vector.tensor_copy(stg[HD:VW, :],
                                              ctx_ps[i][HD:VW, :])
                        cin = ctx_ps[i]
                    else:
                        nc.vector.tensor_copy(stg[0:VW, :],
                                              ctx_ps[i][0:VW, :])
                        cin = stg
                    rb = npool.tile([P, 1024], F32, name=f"rb{i}",
                                    tag=f"rb{i}")
                    rd = npool.tile([P, 1024], F32, name=f"rd{i}",
                                    tag=f"rd{i}")
                    nc.sync.dma_start(rd[0:1, :], stg[HD:VW, :])
                    w = 1
                    while w < 32:
                        nc.sync.dma_start(rd[w:2 * w, :], rd[0:w, :])
                        w *= 2
                    nc.vector.reciprocal_approx_fast(rb[0:32, :],
                                                     rd[0:32, :])
                    nc.sync.dma_start(rb[32:HD, :], rb[0:32, :])
                    if i == 0:
                        nc.vector.tensor_tensor(
                            out=ctxT[p][th][0:HD, :], in0=cin[0:HD, :],
                            in1=rb[0:HD, :], op=ALU.mult)
                    else:
                        ostg = npool.tile([P, 1024], BF16, name="ostg",
                                          tag="ostg")
                        nc.vector.tensor_tensor(
                            out=ostg[0:HD, :], in0=cin[0:HD, :],
                            in1=rb[0:HD, :], op=ALU.mult)
                        nc.sync.dma_start(ctxT[p][th][HD:P, :],
                                          ostg[0:HD, :])

            def emit_scores(p, th, s, i):
                sc = spsum.tile([P, 1024], F32, name=f"sc{i}", tag=f"sc{i}")
                t0 = th * 1024
                ss = slice(s * P, (s + 1) * P)
                hp = slice(i * HD, (i + 1) * HD)
                for c in range(2):
                    nc.tensor.matmul(
                        sc[:, c * 512:(c + 1) * 512], kT[p][hp, ss],
                        qT[p][hp, t0 + c * 512:t0 + (c + 1) * 512],
                        start=True, stop=True)
                return sc

            def emit_ctx(ctx_ps, p, s, i, e):
                h = 2 * p + i
                vs = slice(s * HL * VW + h * VW, s * HL * VW + h * VW + VW)
                for c in range(2):
                    nc.tensor.matmul(
                        ctx_ps[i][:, c * 512:(c + 1) * 512], v_aug[:, vs],
                        e[:, c * 512:(c + 1) * 512],
                        start=(s == 0), stop=(s == ST - 1))

            # software-pipelined, head-serial: the PE stream per step is
            # [scores(s+1,0), ctx(s,0), scores(s+1,1), ctx(s,1)] so no
            # waiting instruction blocks ready work behind it, and the
            # exp stream stays back-to-back on the Scalar engine.
            blocks = ((0, 0), (1, 0), (0, 1), (1, 1))
            sc = [emit_scores(*blocks[0], 0, i) for i in range(2)]
            for bi, (p, th) in enumerate(blocks):
                ctx_ps = [cpsum.tile([VW, 1024], F32, name=f"ctx{i}",
                                     tag=f"ctx{i}") for i in range(2)]
                for s in range(ST):
                    ex = []
                    for i in range(2):
                        e = epool.tile([P, 1024], BF16, name=f"ex{i}",
                                       tag=f"ex{i}")
                        nc.scalar.activation(e[:], sc[i][:], AF.Exp,
                                             scale=0.125)
                        if dbg_d is not None and (p, th, s, i) == (0, 0, 0, 0):
                            nc.sync.dma_start(dbg_d[:, :], e[:])
                        ex.append(e)
                    for i in range(2):
                        if s + 1 < ST:
                            sc[i] = emit_scores(p, th, s + 1, i)
                        elif bi + 1 < len(blocks):
                            sc[i] = emit_scores(*blocks[bi + 1], 0, i)
                        emit_ctx(ctx_ps, p, s, i, ex[i])
                emit_norm(p, th, ctx_ps, last=(bi == len(blocks) - 1))

            # ---- output projection (tail) -----------------------------
            # th=0 tiles evict via the (now idle) Scalar engine so the
            # DVE can finish the last block's normalize concurrently;
            # th=1 tiles alternate DVE/Scalar.  A keepalive matmul burst
            # into the first psum buffer keeps the PE busy through the
            # last normalize (a >3.4us PE idle would re-throttle the
            # clock to 1.2GHz for the whole projection); it is reset by
            # the projection's start=True.
            ka = spsum.tile([P, D], F32, name="ka", tag="sc0")
            for w in range(24):
                nc.tensor.matmul(ka[:, 0:512], ctxT[0][0][:, 0:P],
                                 wo_sb[:, 0:512],
                                 start=(w == 0), stop=(w == 23))
            for t in range(TT):
                th, tl = divmod(t, TT // 2)
                ts = slice(tl * P, (tl + 1) * P)
                if th == 0:
                    ps = spsum.tile([P, D], F32, name="po", tag=f"sc{t % 2}")
                else:
                    tags = ("sc0", "sc1", "ctx0", "ctx1")
                    pool = spsum if t % 4 < 2 else cpsum
                    ps = pool.tile([P, D], F32, name="po", tag=tags[t % 4])
                for p2 in range(2):
                    for n in range(2):
                        ns = slice(n * 512, (n + 1) * 512)
                        nc.tensor.matmul(
                            ps[:, ns], ctxT[p2][th][:, ts],
                            wo_sb[:, p2 * D + n * 512: p2 * D + (n + 1) * 512],
                            start=(p2 == 0), stop=(p2 == 1))
                ost = opool.tile([P, D], BF16, name="ost", tag="ost")
                if th == 0 or t % 2 == 1:
                    nc.scalar.activation(ost[:], ps[:], AF.Copy)
                else:
                    nc.vector.tensor_copy(ost[:], ps[:])
                rings = ((nc.sync, nc.gpsimd) if th == 0
                         else (nc.sync, nc.scalar, nc.gpsimd))
                rings[t % len(rings)].dma_start(
                    out_d[t * P:(t + 1) * P, :], ost[:])


def make_in_maps(query, key, value, Wq, bq, Wk, bk, Wv, bv, Wo, bo):
    """Shard the full inputs into the 8 per-core input dicts."""
    query, key, value, Wq, bq, Wk, bk, Wv, bv, Wo, bo = [
        np.asarray(a, dtype=np.float32)
        for a in (query, key, value, Wq, bq, Wk, bk, Wv, bv, Wo, bo)]

    def bf(a):
        return np.ascontiguousarray(a).astype(BF16_NP)

    in_maps = []
    for c in range(N_CORES):
        b, g = divmod(c, 4)
        sl = slice(g * OUTL, (g + 1) * OUTL)
        kT_ = key[b].T
        vT_ = value[b].T
        in_maps.append({
            "xq": bf(query[b].T),
            "xk0": bf(kT_[:D // 2]), "xk1": bf(kT_[D // 2:]),
            "xv0": bf(vT_[:D // 2]), "xv1": bf(vT_[D // 2:]),
            "wq": bf(Wq[sl, :].T),
            "wk": bf(Wk[sl, :].T),
            "wv": bf(Wv[sl, :].T),
            "wo": bf(Wo[:, sl].T),
            "bias_pack": np.concatenate([
                bq[sl].reshape(2, P).T, bk[sl].reshape(2, P).T,
                np.broadcast_to(bv[sl], (P, OUTL)),
                np.broadcast_to(bv[sl], (P, OUTL))], axis=1).copy(),
        })
    return in_maps


_NC_CACHE = None


def _get_nc():
    global _NC_CACHE
    if _NC_CACHE is None:
        _NC_CACHE = build_program()
    return _NC_CACHE


def gather_out(results, bo):
    out = np.empty((2, T, D), dtype=np.float32)
    bo = np.asarray(bo, dtype=np.float32)
    for b in range(2):
        acc = results[4 * b]["out"].astype(np.float32)
        for g in range(1, 4):
            acc = acc + results[4 * b + g]["out"].astype(np.float32)
        out[b] = acc + bo
    return out


def kernel(query, key, value, Wq, bq, Wk, bk, Wv, bv, Wo, bo):
    nc = _get_nc()
    in_maps = make_in_maps(query, key, value, Wq, bq, Wk, bk, Wv, bv, Wo, bo)
    res = run_bass_kernel_spmd(nc, in_maps, list(range(N_CORES)))
    return gather_out(res.results, bo)
